# revision 12
# baseline (speedup 1.0000x reference)
"""AqlmOFTLinear distributed Trainium2 kernel (8 NeuronCores), v2.

Strategy (pure tensor-parallel over out_features, no collectives):
  - Each core owns 512 out-features: it dequants its own AQLM weight shard,
    rotates it (V = Q_blockdiag @ W^T) into SBUF, and computes
    out^T[o_shard, t] for ALL 16384 tokens.  Host concatenates shards.
  - AQLM dequant via gpsimd.dma_gather from a paired codebook table
    [32768, 128]bf16 (entry q = [cb[q] | cb[q+32768]]); int15 index.
    32 calls x 8192 idx, one SWDGE queue per call in oc-major bursts of
    4 -- the 4 queues generate descriptors in parallel (~64us/call,
    ~82us/burst), so V[oc] lands every ~165us and the whole gather
    stream finishes in ~660us.
  - The gather stream order is chosen so the (o, group) -> (i'', o)
    transpose needed for the rotation matmul is exactly a DVE 32x32
    stream-transpose (InstStreamTranspose) -- no PE transposes at all.
    Select+scale is one DVE multiply with host-built coefficients
    (lo: s*(1-m), hi: s*m) and the lo+hi sum is one DVE add.
  - OFT Cayley on-device: Q^T = (I+S^8)(I+S^4)(I+S^2)(I-S)^2 with the 128
    32x32 blocks packed 4-up into block-diagonal 128x128 f32 matmuls.
  - Main matmul in bf16, back-to-back 512-wide matmuls (measured ~260ns
    sustained, LDWEIGHTS hidden): PSUM holds 2 o-blocks x 1024 tokens;
    x is streamed from DRAM twice (once per o-block pair).  Bias fused
    into the PSUM->SBUF evacuation; bf16 output.
"""

import os
import sys

import numpy as np

sys.path.insert(0, "/opt/trn_rl_repo")

import ml_dtypes

BF16 = ml_dtypes.bfloat16

N_CORES = 8
IN_F = 4096
OUT_F = 4096
TOK = 16384
OUT_PC = OUT_F // N_CORES        # 512 out-features per core
GROUP = 8
N_IC = IN_F // 128               # 32 input-feature chunks
HALF_CB = 32768                  # paired table entries
ELEM = 128                       # bf16 elems per table entry (256B)
N_CALLS = 32                     # gather calls per core: (oc, j)
NIDX = 8192                      # indices per gather call (4 ic-chunks)
NQ = 4                           # SWDGE queues
TCH = 1024                       # tokens per main-GEMM chunk
N_CH = TOK // TCH                # 16 chunks per pass

_BUILD_CACHE = {}
LAST_RESULT = None


def _patched_dma_gather():
    """dma_gather with the elem_size %256 assert relaxed: the 256B constraint
    is xbar-transpose-only; natural-mode 32B elements work on HW (verified)
    and cut gather traffic 8x by skipping the table pad."""
    import inspect
    import re

    import concourse.bass as cb

    fsrc = inspect.getsource(type(cb.Bass().gpsimd).dma_gather)
    fsrc = fsrc.replace(
        "elem_size_bytes > 0 and elem_size_bytes % 256 == 0", "elem_size_bytes > 0"
    )
    fsrc = re.sub(r"^    def dma_gather", "def dma_gather", fsrc, flags=re.M)
    fsrc = re.sub(r"\n    ", "\n", fsrc)
    ns = dict(vars(cb))
    exec(compile(fsrc, "patched_dma_gather", "exec"), ns)
    return ns["dma_gather"]


def _build_nc():
    from concourse import bacc, mybir, tile

    dma_gather32 = _patched_dma_gather()

    f32 = mybir.dt.float32
    bf16 = mybir.dt.bfloat16
    i16 = mybir.dt.int16

    nc = bacc.Bacc(num_devices=N_CORES, num_swdge_queues=NQ)

    # ---- DRAM parameters ----
    xT_d = nc.declare_dram_parameter("xT", [IN_F, TOK], bf16, isOutput=False)
    table_d = nc.declare_dram_parameter("table", [HALF_CB, ELEM], bf16, isOutput=False)
    idx_d = nc.declare_dram_parameter("idx", [N_CALLS, 128, NIDX // 16], i16, isOutput=False)
    abt_d = nc.declare_dram_parameter("abt", [N_CALLS, 128, 1024], bf16, isOutput=False)
    rbd_d = nc.declare_dram_parameter("rbd", [N_IC, 128, 128], f32, isOutput=False)
    identf_d = nc.declare_dram_parameter("identf", [128, 128], f32, isOutput=False)
    bias_d = nc.declare_dram_parameter("bias_p", [128, 4], f32, isOutput=False)
    outT_d = nc.declare_dram_parameter("outT", [OUT_PC, TOK], bf16, isOutput=True)
    DBG = bool(int(os.environ.get("AQLM_DEBUG", "0")))
    if DBG:
        dbg_qt_d = nc.declare_dram_parameter("dbg_qt", [128, N_IC, 128], bf16, isOutput=True)
        dbg_v_d = nc.declare_dram_parameter("dbg_v", [128, N_IC, OUT_PC], bf16, isOutput=True)
        dbg_g_d = nc.declare_dram_parameter("dbg_g", [128, 64, 16], bf16, isOutput=True)
        dbg_gs_d = nc.declare_dram_parameter("dbg_gs", [128, 64, 16], bf16, isOutput=True)
        dbg_w4_d = nc.declare_dram_parameter("dbg_w4", [128, 512], bf16, isOutput=True)
        dbg_wt_d = nc.declare_dram_parameter("dbg_wt", [128, 512], bf16, isOutput=True)

    with tile.TileContext(nc) as tc:
        with (
            tc.tile_pool(name="const", bufs=1) as constp,
            tc.tile_pool(name="qt", bufs=1) as qtp,
            tc.tile_pool(name="vsb", bufs=1) as vp,
            tc.tile_pool(name="xh", bufs=5) as xhp,
            tc.tile_pool(name="cay", bufs=4) as cayp,
            tc.tile_pool(name="idx", bufs=12) as idxp,
            tc.tile_pool(name="abt", bufs=10) as abtp,
            tc.tile_pool(name="g", bufs=6) as gp,
            tc.tile_pool(name="gs", bufs=2) as gsp,
            tc.tile_pool(name="w4", bufs=2) as w4p,
            tc.tile_pool(name="wt", bufs=4) as wtp,
            tc.tile_pool(name="ob", bufs=4) as obp,
            tc.tile_pool(name="psm", bufs=2, space="PSUM") as psmp,
            tc.tile_pool(name="pss", bufs=4, space="PSUM") as pssp,
        ):
            # ---- constants (sync queue) ----
            identf = constp.tile([128, 128], f32)
            nc.sync.dma_start(out=identf[:], in_=identf_d[:])
            bias_sb = constp.tile([128, 4], f32)
            nc.sync.dma_start(out=bias_sb[:], in_=bias_d[:])
            ident4 = constp.tile([128, 4, 128], f32)
            for k in range(4):
                nc.vector.tensor_copy(ident4[:, k, :], identf[:])

            qt_sb = qtp.tile([128, N_IC, 128], bf16)   # Q^T block-diag chunks
            V_sb = vp.tile([128, N_IC, OUT_PC], bf16)  # rotated weight shard
            nidx_reg = nc.gpsimd.to_reg(NIDX)

            # ---- idx/abt prefetch + gathers (sync + gpsimd queues) ----
            g_tiles = []
            for k in range(N_CALLS):
                idx_sb = idxp.tile([128, NIDX // 16], i16, tag="idx")
                nc.scalar.dma_start(out=idx_sb[:], in_=idx_d[k, :, :])
                abt_sb = abtp.tile([128, 64, 16], bf16, tag="abt")
                nc.scalar.dma_start(
                    out=abt_sb[:],
                    in_=abt_d[k, :, :].rearrange("p (c e) -> p c e", e=16),
                )
                G = gp.tile([128, 64, 16], bf16, tag="G")
                dma_gather32(
                    nc.gpsimd, G[:], table_d[:, 0:16], idx_sb[:],
                    num_idxs=NIDX, num_idxs_reg=nidx_reg,
                    elem_size=16, elem_step=ELEM,
                    single_packet=False, queue_num=k % NQ,
                )
                if DBG and k == 0:
                    nc.sync.dma_start(out=dbg_g_d[:], in_=G[:])
                g_tiles.append((G, abt_sb))

            # ================= Cayley (PE + DVE, head) =================
            for g in range(8):
                rbd_sb = cayp.tile([128, 4, 128], f32, tag="cay")
                nc.sync.dma_start(
                    out=rbd_sb[:],
                    in_=rbd_d[g * 4:(g + 1) * 4, :, :].rearrange("c p f -> p c f"),
                )
                psT = pssp.tile([128, 4, 128], f32, tag="sm")
                for k in range(4):
                    nc.tensor.transpose(psT[:, k, :], rbd_sb[:, k, :], identf[:])
                tmp = cayp.tile([128, 4, 128], f32, tag="cay")
                nc.vector.tensor_scalar_mul(tmp[:], rbd_sb[:], 0.5)
                S = cayp.tile([128, 4, 128], f32, tag="cay")
                nc.vector.scalar_tensor_tensor(
                    S[:], psT[:], -0.5, tmp[:],
                    mybir.AluOpType.mult, mybir.AluOpType.add,
                )
                negS = cayp.tile([128, 4, 128], f32, tag="cay")
                nc.vector.tensor_scalar_mul(negS[:], S[:], -1.0)
                P1T = cayp.tile([128, 4, 128], f32, tag="cay")  # I - S
                nc.vector.scalar_tensor_tensor(
                    P1T[:], S[:], -1.0, ident4[:],
                    mybir.AluOpType.mult, mybir.AluOpType.add,
                )
                P1 = cayp.tile([128, 4, 128], f32, tag="cay")  # I + S
                nc.vector.tensor_tensor(P1[:], S[:], ident4[:], mybir.AluOpType.add)
                ps2 = pssp.tile([128, 4, 128], f32, tag="sm")
                for k in range(4):
                    nc.tensor.matmul(ps2[:, k, :], negS[:, k, :], S[:, k, :])
                S2 = cayp.tile([128, 4, 128], f32, tag="cay")
                nc.vector.tensor_copy(S2[:], ps2[:])
                P2 = cayp.tile([128, 4, 128], f32, tag="cay")  # I + S^2
                nc.vector.tensor_tensor(P2[:], S2[:], ident4[:], mybir.AluOpType.add)
                ps4 = pssp.tile([128, 4, 128], f32, tag="sm")
                for k in range(4):
                    nc.tensor.matmul(ps4[:, k, :], S2[:, k, :], S2[:, k, :])
                S4 = cayp.tile([128, 4, 128], f32, tag="cay")
                nc.vector.tensor_copy(S4[:], ps4[:])
                P3 = cayp.tile([128, 4, 128], f32, tag="cay")  # I + S^4
                nc.vector.tensor_tensor(P3[:], S4[:], ident4[:], mybir.AluOpType.add)
                ps8 = pssp.tile([128, 4, 128], f32, tag="sm")
                for k in range(4):
                    nc.tensor.matmul(ps8[:, k, :], S4[:, k, :], S4[:, k, :])
                P4 = cayp.tile([128, 4, 128], f32, tag="cay")  # I + S^8
                nc.vector.scalar_tensor_tensor(
                    P4[:], ps8[:], 1.0, ident4[:],
                    mybir.AluOpType.mult, mybir.AluOpType.add,
                )
                psT1 = pssp.tile([128, 4, 128], f32, tag="sm")
                for k in range(4):
                    nc.tensor.matmul(psT1[:, k, :], P1[:, k, :], P1T[:, k, :])
                T1 = cayp.tile([128, 4, 128], f32, tag="cay")  # (I-S)^2
                nc.vector.tensor_copy(T1[:], psT1[:])
                psb1 = pssp.tile([128, 4, 128], f32, tag="sm")
                for k in range(4):
                    nc.tensor.matmul(psb1[:, k, :], P2[:, k, :], T1[:, k, :])
                B1 = cayp.tile([128, 4, 128], f32, tag="cay")
                nc.vector.tensor_copy(B1[:], psb1[:])
                psb2 = pssp.tile([128, 4, 128], f32, tag="sm")
                for k in range(4):
                    nc.tensor.matmul(psb2[:, k, :], P3[:, k, :], B1[:, k, :])
                B2 = cayp.tile([128, 4, 128], f32, tag="cay")
                nc.vector.tensor_copy(B2[:], psb2[:])
                psb3 = pssp.tile([128, 4, 128], f32, tag="sm")
                for k in range(4):
                    nc.tensor.matmul(psb3[:, k, :], P4[:, k, :], B2[:, k, :])
                nc.vector.tensor_copy(qt_sb[:, g * 4:(g + 1) * 4, :], psb3[:])

            # ---- dequant per call: DVE mul/add/stream-transpose, PE rot ----
            def emit_dequant(k):
                G, abt_sb = g_tiles[k]
                Gs = gsp.tile([128, 64, 16], bf16, tag="gs")
                nc.vector.tensor_tensor(Gs[:], G[:], abt_sb[:], mybir.AluOpType.mult)
                # c-slot mapping (s, oh, gm) makes the lo+hi add output plain
                # contiguous; the stream-transpose then sees [p, 16 tiles, 32]
                # (3-dim AP limit) and lands W^T tiles [i''-local, o-local].
                W4 = w4p.tile([128, 512], bf16, tag="w4")
                nc.vector.tensor_tensor(
                    W4[:].rearrange("p (c j) -> p c j", j=8),
                    Gs[:, :, 0:8], Gs[:, :, 8:16], mybir.AluOpType.add,
                )
                wt = wtp.tile([128, 512], bf16, tag="wt")
                nc.vector.transpose(
                    wt[:].rearrange("p (t f) -> p t f", f=32),
                    W4[:].rearrange("p (t f) -> p t f", f=32),
                )
                if DBG and k == 0:
                    nc.sync.dma_start(out=dbg_gs_d[:], in_=Gs[:])
                    nc.sync.dma_start(out=dbg_w4_d[:], in_=W4[:])
                    nc.sync.dma_start(out=dbg_wt_d[:], in_=wt[:])
                return wt

            def emit_rot(k, wt):
                oc, j = k // 8, k % 8
                psv = pssp.tile([128, 4, 128], f32, tag="sm")
                for s in range(4):
                    ic = 4 * j + s
                    nc.tensor.matmul(
                        psv[:, s, :], qt_sb[:, ic, :], wt[:, s * 128:(s + 1) * 128],
                        start=True, stop=True,
                    )
                nc.vector.tensor_copy(
                    V_sb[:, 4 * j:4 * j + 4, oc * 128:(oc + 1) * 128], psv[:]
                )

            # dequant oc0+oc1 fully (gathers land ~190us / ~355us)
            wts = {}
            for k in range(16):
                wts[k] = emit_dequant(k)
                emit_rot(k, wts[k])

            # ---- main GEMM: two passes of (2 o-blocks x 16 chunks) ----
            def emit_chunk(ocA, ocB, c, xh_quarters):
                for obi, oc in ((0, ocA), (1, ocB)):
                    psm = psmp.tile([128, TCH], f32, tag="m")
                    for ic in range(N_IC):
                        xh = xh_quarters[ic // 8]
                        for h2 in range(2):
                            nc.tensor.matmul(
                                psm[:, h2 * 512:(h2 + 1) * 512],
                                V_sb[:, ic, oc * 128:(oc + 1) * 128],
                                xh[:, ic % 8, h2 * 512:(h2 + 1) * 512],
                                start=(ic == 0), stop=(ic == N_IC - 1),
                            )
                    ob = obp.tile([128, TCH], bf16, tag="ob")
                    nc.vector.tensor_scalar_add(
                        ob[:], psm[:], bias_sb[:, oc:oc + 1]
                    )
                    nc.sync.dma_start(
                        out=outT_d[oc * 128:(oc + 1) * 128, c * TCH:(c + 1) * TCH],
                        in_=ob[:],
                    )

            for p, (ocA, ocB) in enumerate(((0, 1), (2, 3))):
                for c in range(N_CH):
                    quarters = []
                    for ih in range(4):
                        xh = xhp.tile([128, 8, TCH], bf16, tag="xh")
                        nc.sync.dma_start(
                            out=xh[:],
                            in_=xT_d[
                                ih * 1024:(ih + 1) * 1024, c * TCH:(c + 1) * TCH
                            ].rearrange("(ic q) t -> q ic t", q=128),
                        )
                        quarters.append(xh)
                    emit_chunk(ocA, ocB, c, quarters)
                    # weave oc2/oc3 dequant behind early pass-1 chunks
                    if p == 0 and c in (5, 10):
                        base = 16 if c == 5 else 24
                        for k in range(base, base + 8):
                            wts[k] = emit_dequant(k)
                            emit_rot(k, wts[k])
            if DBG:
                nc.sync.dma_start(out=dbg_qt_d[:], in_=qt_sb[:])
                nc.sync.dma_start(out=dbg_v_d[:], in_=V_sb[:])
    nc.compile()
    return nc


def _host_prep(x, oft_r, codes, codebooks, scales, bias):
    """Shard + repack all inputs for the 8 cores."""
    xT = np.ascontiguousarray(
        np.asarray(x, dtype=np.float32).reshape(TOK, IN_F).astype(BF16).T
    )                                                           # [4096, 16384]
    codes2 = np.asarray(codes, dtype=np.int64)[:, :, 0]         # [4096, 512]
    cb = np.asarray(codebooks, dtype=np.float32)[0]             # [65536, 8]
    scales = np.asarray(scales, dtype=np.float32).reshape(OUT_F)
    bias = np.asarray(bias, dtype=np.float32).reshape(OUT_F)
    R = np.asarray(oft_r, dtype=np.float32)                     # [128, 32, 32]

    table = np.zeros((HALF_CB, ELEM), dtype=BF16)
    table[:, 0:GROUP] = cb[:HALF_CB].astype(BF16)
    table[:, GROUP:2 * GROUP] = cb[HALF_CB:].astype(BF16)

    rbd = np.zeros((N_IC, 128, 128), dtype=np.float32)
    Rb = R.reshape(N_IC, 4, 32, 32)
    for a in range(4):
        rbd[:, a * 32:(a + 1) * 32, a * 32:(a + 1) * 32] = Rb[:, a]
    identf = np.eye(128, dtype=np.float32)

    idx14 = (codes2 & 32767).astype(np.int16)
    mfull = (codes2 >> 15).astype(np.float32)

    s_g, gl_g, o_g = np.meshgrid(
        np.arange(4), np.arange(16), np.arange(128), indexing="ij"
    )
    c_gm = s_g * 16 + (o_g >> 5) * 4 + (gl_g & 3)
    p_gm = (gl_g >> 2) * 32 + (o_g & 31)
    n_g = (c_gm * 128 + p_gm).ravel()
    p_g = p_gm.ravel()
    c_g = c_gm.ravel()

    in_maps = []
    for r in range(N_CORES):
        idx_all = np.empty((N_CALLS, 128, NIDX // 16), dtype=np.int16)
        abt_all = np.empty((N_CALLS, 128, 64, 16), dtype=BF16)
        for k in range(N_CALLS):
            oc, j = k // 8, k % 8
            ic = 4 * j + s_g
            g = ic * 16 + gl_g
            o_glob = r * OUT_PC + oc * 128 + o_g
            vals = idx14[o_glob, g]
            stream = np.empty(NIDX, dtype=np.int16)
            stream[n_g] = vals.ravel()
            idx_all[k] = np.broadcast_to(
                stream.reshape(NIDX // 16, 16).T[None, :, :], (8, 16, NIDX // 16)
            ).reshape(128, NIDX // 16)
            sc = scales[o_glob]
            B = sc * mfull[o_glob, g]
            A = sc - B
            ab = np.empty((128, 64, 16), dtype=np.float32)
            ab[p_g, c_g, 0:8] = A.ravel()[:, None]
            ab[p_g, c_g, 8:16] = B.ravel()[:, None]
            abt_all[k] = ab.astype(BF16)
        bias_p = np.zeros((128, 4), dtype=np.float32)
        for oc in range(4):
            bias_p[:, oc] = bias[r * OUT_PC + oc * 128:r * OUT_PC + (oc + 1) * 128]
        in_maps.append(
            dict(
                xT=xT,
                table=table,
                idx=idx_all,
                abt=abt_all.reshape(N_CALLS, 128, 1024),
                rbd=rbd,
                identf=identf,
                bias_p=bias_p,
            )
        )
    return in_maps


def kernel(x, oft_r, codes, codebooks, scales, bias):
    global LAST_RESULT
    from concourse.bass_utils import run_bass_kernel_spmd

    if "nc" not in _BUILD_CACHE:
        _BUILD_CACHE["nc"] = _build_nc()
    nc = _BUILD_CACHE["nc"]

    in_maps = _host_prep(x, oft_r, codes, codebooks, scales, bias)
    trace = bool(int(os.environ.get("AQLM_TRACE", "0")))
    res = run_bass_kernel_spmd(nc, in_maps, core_ids=list(range(N_CORES)), trace=trace)
    LAST_RESULT = res

    out = np.empty((TOK, OUT_F), dtype=np.float32)
    for r in range(N_CORES):
        out[:, r * OUT_PC:(r + 1) * OUT_PC] = (
            res.results[r]["outT"].T.astype(np.float32)
        )
    return out.reshape(4, 4096, 4096).astype(np.asarray(x).dtype)


# revision 13
# speedup vs baseline: 1.1589x; 1.1589x over previous
"""AqlmOFTLinear distributed Trainium2 kernel (8 NeuronCores), v2.

Strategy (pure tensor-parallel over out_features, no collectives):
  - Each core owns 512 out-features: it dequants its own AQLM weight shard,
    rotates it (V = Q_blockdiag @ W^T) into SBUF, and computes
    out^T[o_shard, t] for ALL 16384 tokens.  Host concatenates shards.
  - AQLM dequant via gpsimd.dma_gather from a paired codebook table
    [32768, 128]bf16 (entry q = [cb[q] | cb[q+32768]]); int15 index.
    32 calls x 8192 idx, one SWDGE queue per call in oc-major bursts of
    4 -- the 4 queues generate descriptors in parallel (~64us/call,
    ~82us/burst), so V[oc] lands every ~165us and the whole gather
    stream finishes in ~660us.
  - The gather stream order is chosen so the (o, group) -> (i'', o)
    transpose needed for the rotation matmul is exactly a DVE 32x32
    stream-transpose (InstStreamTranspose) -- no PE transposes at all.
    Select+scale is one DVE multiply with host-built coefficients
    (lo: s*(1-m), hi: s*m) and the lo+hi sum is one DVE add.
  - OFT Cayley on-device: Q^T = (I+S^8)(I+S^4)(I+S^2)(I-S)^2 with the 128
    32x32 blocks packed 4-up into block-diagonal 128x128 f32 matmuls.
  - Main matmul in bf16, back-to-back 512-wide matmuls (measured ~260ns
    sustained, LDWEIGHTS hidden): PSUM holds 2 o-blocks x 1024 tokens;
    x is streamed from DRAM twice (once per o-block pair).  Bias fused
    into the PSUM->SBUF evacuation; bf16 output.
"""

import os
import sys

import numpy as np

sys.path.insert(0, "/opt/trn_rl_repo")

import ml_dtypes

BF16 = ml_dtypes.bfloat16

N_CORES = 8
IN_F = 4096
OUT_F = 4096
TOK = 16384
OUT_PC = OUT_F // N_CORES        # 512 out-features per core
GROUP = 8
N_IC = IN_F // 128               # 32 input-feature chunks
HALF_CB = 32768                  # paired table entries
ELEM = 128                       # bf16 elems per table entry (256B)
N_CALLS = 32                     # gather calls per core: (oc, j)
NIDX = 8192                      # indices per gather call (4 ic-chunks)
NQ = 4                           # SWDGE queues
TCH = 1024                       # tokens per main-GEMM chunk
N_CH = TOK // TCH                # 16 chunks per pass

_BUILD_CACHE = {}
LAST_RESULT = None


def _patched_dma_gather():
    """dma_gather with the elem_size %256 assert relaxed: the 256B constraint
    is xbar-transpose-only; natural-mode 32B elements work on HW (verified)
    and cut gather traffic 8x by skipping the table pad."""
    import inspect
    import re

    import concourse.bass as cb

    fsrc = inspect.getsource(type(cb.Bass().gpsimd).dma_gather)
    fsrc = fsrc.replace(
        "elem_size_bytes > 0 and elem_size_bytes % 256 == 0", "elem_size_bytes > 0"
    )
    fsrc = re.sub(r"^    def dma_gather", "def dma_gather", fsrc, flags=re.M)
    fsrc = re.sub(r"\n    ", "\n", fsrc)
    ns = dict(vars(cb))
    exec(compile(fsrc, "patched_dma_gather", "exec"), ns)
    return ns["dma_gather"]


def _build_nc():
    from concourse import bacc, mybir, tile

    dma_gather32 = _patched_dma_gather()

    f32 = mybir.dt.float32
    bf16 = mybir.dt.bfloat16
    i16 = mybir.dt.int16

    nc = bacc.Bacc(num_devices=N_CORES, num_swdge_queues=NQ)

    # ---- DRAM parameters ----
    xT_d = nc.declare_dram_parameter("xT", [IN_F, TOK], bf16, isOutput=False)
    table_d = nc.declare_dram_parameter("table", [HALF_CB, ELEM], bf16, isOutput=False)
    idx_d = nc.declare_dram_parameter("idx", [N_CALLS, 128, NIDX // 16], i16, isOutput=False)
    abt_d = nc.declare_dram_parameter("abt", [N_CALLS, 128, 1024], bf16, isOutput=False)
    rbd_d = nc.declare_dram_parameter("rbd", [N_IC, 128, 128], f32, isOutput=False)
    identf_d = nc.declare_dram_parameter("identf", [128, 128], f32, isOutput=False)
    bias_d = nc.declare_dram_parameter("bias_p", [128, 4], f32, isOutput=False)
    outT_d = nc.declare_dram_parameter("outT", [OUT_PC, TOK], bf16, isOutput=True)
    DBG = bool(int(os.environ.get("AQLM_DEBUG", "0")))
    if DBG:
        dbg_qt_d = nc.declare_dram_parameter("dbg_qt", [128, N_IC, 128], bf16, isOutput=True)
        dbg_v_d = nc.declare_dram_parameter("dbg_v", [128, N_IC, OUT_PC], bf16, isOutput=True)
        dbg_g_d = nc.declare_dram_parameter("dbg_g", [128, 64, 16], bf16, isOutput=True)
        dbg_gs_d = nc.declare_dram_parameter("dbg_gs", [128, 64, 16], bf16, isOutput=True)
        dbg_w4_d = nc.declare_dram_parameter("dbg_w4", [128, 512], bf16, isOutput=True)
        dbg_wt_d = nc.declare_dram_parameter("dbg_wt", [128, 512], bf16, isOutput=True)

    with tile.TileContext(nc) as tc:
        with (
            tc.tile_pool(name="const", bufs=1) as constp,
            tc.tile_pool(name="qt", bufs=1) as qtp,
            tc.tile_pool(name="vsb", bufs=1) as vp,
            tc.tile_pool(name="xh", bufs=5) as xhp,
            tc.tile_pool(name="cay", bufs=4) as cayp,
            tc.tile_pool(name="idx", bufs=12) as idxp,
            tc.tile_pool(name="abt", bufs=10) as abtp,
            tc.tile_pool(name="g", bufs=8) as gp,
            tc.tile_pool(name="gs", bufs=2) as gsp,
            tc.tile_pool(name="w4", bufs=2) as w4p,
            tc.tile_pool(name="wt", bufs=4) as wtp,
            tc.tile_pool(name="ob", bufs=4) as obp,
            tc.tile_pool(name="psm", bufs=3, space="PSUM") as psmp,
            tc.tile_pool(name="pss", bufs=2, space="PSUM") as pssp,
        ):
            # ---- constants (sync queue) ----
            identf = constp.tile([128, 128], f32)
            nc.sync.dma_start(out=identf[:], in_=identf_d[:])
            bias_sb = constp.tile([128, 4], f32)
            nc.sync.dma_start(out=bias_sb[:], in_=bias_d[:])
            ident4 = constp.tile([128, 4, 128], f32)
            for k in range(4):
                nc.vector.tensor_copy(ident4[:, k, :], identf[:])

            qt_sb = qtp.tile([128, N_IC, 128], bf16)   # Q^T block-diag chunks
            V_sb = vp.tile([128, N_IC, OUT_PC], bf16)  # rotated weight shard
            nidx_reg = nc.gpsimd.to_reg(NIDX)

            # ---- idx/abt prefetch + gathers (sync + gpsimd queues) ----
            g_tiles = []
            for k in range(N_CALLS):
                idx_sb = idxp.tile([128, NIDX // 16], i16, tag="idx")
                nc.scalar.dma_start(out=idx_sb[:], in_=idx_d[k, :, :])
                abt_sb = abtp.tile([128, 64, 16], bf16, tag="abt")
                nc.scalar.dma_start(
                    out=abt_sb[:],
                    in_=abt_d[k, :, :].rearrange("p (c e) -> p c e", e=16),
                )
                G = gp.tile([128, 64, 16], bf16, tag="G")
                dma_gather32(
                    nc.gpsimd, G[:], table_d[:, 0:16], idx_sb[:],
                    num_idxs=NIDX, num_idxs_reg=nidx_reg,
                    elem_size=16, elem_step=ELEM,
                    single_packet=False, queue_num=k % NQ,
                )
                if DBG and k == 0:
                    nc.sync.dma_start(out=dbg_g_d[:], in_=G[:])
                g_tiles.append((G, abt_sb))

            # ================= Cayley (PE + DVE, head) =================
            for g in range(8):
                rbd_sb = cayp.tile([128, 4, 128], f32, tag="cay")
                nc.sync.dma_start(
                    out=rbd_sb[:],
                    in_=rbd_d[g * 4:(g + 1) * 4, :, :].rearrange("c p f -> p c f"),
                )
                psT = pssp.tile([128, 4, 128], f32, tag="sm")
                for k in range(4):
                    nc.tensor.transpose(psT[:, k, :], rbd_sb[:, k, :], identf[:])
                tmp = cayp.tile([128, 4, 128], f32, tag="cay")
                nc.vector.tensor_scalar_mul(tmp[:], rbd_sb[:], 0.5)
                S = cayp.tile([128, 4, 128], f32, tag="cay")
                nc.vector.scalar_tensor_tensor(
                    S[:], psT[:], -0.5, tmp[:],
                    mybir.AluOpType.mult, mybir.AluOpType.add,
                )
                negS = cayp.tile([128, 4, 128], f32, tag="cay")
                nc.vector.tensor_scalar_mul(negS[:], S[:], -1.0)
                P1T = cayp.tile([128, 4, 128], f32, tag="cay")  # I - S
                nc.vector.scalar_tensor_tensor(
                    P1T[:], S[:], -1.0, ident4[:],
                    mybir.AluOpType.mult, mybir.AluOpType.add,
                )
                P1 = cayp.tile([128, 4, 128], f32, tag="cay")  # I + S
                nc.vector.tensor_tensor(P1[:], S[:], ident4[:], mybir.AluOpType.add)
                ps2 = pssp.tile([128, 4, 128], f32, tag="sm")
                for k in range(4):
                    nc.tensor.matmul(ps2[:, k, :], negS[:, k, :], S[:, k, :])
                S2 = cayp.tile([128, 4, 128], f32, tag="cay")
                nc.vector.tensor_copy(S2[:], ps2[:])
                P2 = cayp.tile([128, 4, 128], f32, tag="cay")  # I + S^2
                nc.vector.tensor_tensor(P2[:], S2[:], ident4[:], mybir.AluOpType.add)
                ps4 = pssp.tile([128, 4, 128], f32, tag="sm")
                for k in range(4):
                    nc.tensor.matmul(ps4[:, k, :], S2[:, k, :], S2[:, k, :])
                S4 = cayp.tile([128, 4, 128], f32, tag="cay")
                nc.vector.tensor_copy(S4[:], ps4[:])
                P3 = cayp.tile([128, 4, 128], f32, tag="cay")  # I + S^4
                nc.vector.tensor_tensor(P3[:], S4[:], ident4[:], mybir.AluOpType.add)
                ps8 = pssp.tile([128, 4, 128], f32, tag="sm")
                for k in range(4):
                    nc.tensor.matmul(ps8[:, k, :], S4[:, k, :], S4[:, k, :])
                P4 = cayp.tile([128, 4, 128], f32, tag="cay")  # I + S^8
                nc.vector.scalar_tensor_tensor(
                    P4[:], ps8[:], 1.0, ident4[:],
                    mybir.AluOpType.mult, mybir.AluOpType.add,
                )
                psT1 = pssp.tile([128, 4, 128], f32, tag="sm")
                for k in range(4):
                    nc.tensor.matmul(psT1[:, k, :], P1[:, k, :], P1T[:, k, :])
                T1 = cayp.tile([128, 4, 128], f32, tag="cay")  # (I-S)^2
                nc.vector.tensor_copy(T1[:], psT1[:])
                psb1 = pssp.tile([128, 4, 128], f32, tag="sm")
                for k in range(4):
                    nc.tensor.matmul(psb1[:, k, :], P2[:, k, :], T1[:, k, :])
                B1 = cayp.tile([128, 4, 128], f32, tag="cay")
                nc.vector.tensor_copy(B1[:], psb1[:])
                psb2 = pssp.tile([128, 4, 128], f32, tag="sm")
                for k in range(4):
                    nc.tensor.matmul(psb2[:, k, :], P3[:, k, :], B1[:, k, :])
                B2 = cayp.tile([128, 4, 128], f32, tag="cay")
                nc.vector.tensor_copy(B2[:], psb2[:])
                psb3 = pssp.tile([128, 4, 128], f32, tag="sm")
                for k in range(4):
                    nc.tensor.matmul(psb3[:, k, :], P4[:, k, :], B2[:, k, :])
                nc.vector.tensor_copy(qt_sb[:, g * 4:(g + 1) * 4, :], psb3[:])

            # ---- dequant per call: DVE mul/add/stream-transpose, PE rot ----
            def emit_dequant(k):
                G, abt_sb = g_tiles[k]
                Gs = gsp.tile([128, 64, 16], bf16, tag="gs")
                nc.vector.tensor_tensor(Gs[:], G[:], abt_sb[:], mybir.AluOpType.mult)
                # c-slot mapping (s, oh, gm) makes the lo+hi add output plain
                # contiguous; the stream-transpose then sees [p, 16 tiles, 32]
                # (3-dim AP limit) and lands W^T tiles [i''-local, o-local].
                W4 = w4p.tile([128, 512], bf16, tag="w4")
                nc.vector.tensor_tensor(
                    W4[:].rearrange("p (c j) -> p c j", j=8),
                    Gs[:, :, 0:8], Gs[:, :, 8:16], mybir.AluOpType.add,
                )
                wt = wtp.tile([128, 512], bf16, tag="wt")
                nc.vector.transpose(
                    wt[:].rearrange("p (t f) -> p t f", f=32),
                    W4[:].rearrange("p (t f) -> p t f", f=32),
                )
                if DBG and k == 0:
                    nc.sync.dma_start(out=dbg_gs_d[:], in_=Gs[:])
                    nc.sync.dma_start(out=dbg_w4_d[:], in_=W4[:])
                    nc.sync.dma_start(out=dbg_wt_d[:], in_=wt[:])
                return wt

            def emit_rot(k, wt):
                oc, j = k // 8, k % 8
                psv = pssp.tile([128, 4, 128], f32, tag="sm")
                for s in range(4):
                    ic = 4 * j + s
                    nc.tensor.matmul(
                        psv[:, s, :], qt_sb[:, ic, :], wt[:, s * 128:(s + 1) * 128],
                        start=True, stop=True,
                    )
                nc.vector.tensor_copy(
                    V_sb[:, 4 * j:4 * j + 4, oc * 128:(oc + 1) * 128], psv[:]
                )

            # DVE dequant ops for ALL calls up-front (gathers pace them);
            # PE rotations are woven into the main-GEMM stream instead.
            wts = {}
            for k in range(N_CALLS):
                wts[k] = emit_dequant(k)
            for k in range(8):
                emit_rot(k, wts[k])

            # ---- main GEMM: two passes of (2 o-blocks x 16 chunks) ----
            def emit_chunk(ocA, ocB, c, xh_quarters, only=None):
                obs = ((0, ocA), (1, ocB))
                if only is not None:
                    obs = (obs[only],)
                for obi, oc in obs:
                    psm = psmp.tile([128, TCH], f32, tag="m")
                    for ic in range(N_IC):
                        xh = xh_quarters[ic // 8]
                        for h2 in range(2):
                            nc.tensor.matmul(
                                psm[:, h2 * 512:(h2 + 1) * 512],
                                V_sb[:, ic, oc * 128:(oc + 1) * 128],
                                xh[:, ic % 8, h2 * 512:(h2 + 1) * 512],
                                start=(ic == 0), stop=(ic == N_IC - 1),
                            )
                    ob = obp.tile([128, TCH], bf16, tag="ob")
                    nc.scalar.activation(
                        ob[:], psm[:], mybir.ActivationFunctionType.Identity,
                        bias=bias_sb[:, oc:oc + 1],
                    )
                    nc.sync.dma_start(
                        out=outT_d[oc * 128:(oc + 1) * 128, c * TCH:(c + 1) * TCH],
                        in_=ob[:],
                    )

            for p, (ocA, ocB) in enumerate(((0, 1), (2, 3))):
                for c in range(N_CH):
                    quarters = []
                    for ih in range(4):
                        xh = xhp.tile([128, 8, TCH], bf16, tag="xh")
                        nc.sync.dma_start(
                            out=xh[:],
                            in_=xT_d[
                                ih * 1024:(ih + 1) * 1024, c * TCH:(c + 1) * TCH
                            ].rearrange("(ic q) t -> q ic t", q=128),
                        )
                        quarters.append(xh)
                    if p == 0 and c == 0:
                        # obA first so it is not blocked behind oc1 rotations
                        emit_chunk(ocA, ocB, c, quarters, only=0)
                        for k in range(8, 16):
                            emit_rot(k, wts[k])
                        emit_chunk(ocA, ocB, c, quarters, only=1)
                    else:
                        emit_chunk(ocA, ocB, c, quarters)
                    # weave oc2/oc3 PE rotations behind early pass-1 chunks
                    if p == 0 and c in (4, 9):
                        base = 16 if c == 4 else 24
                        for k in range(base, base + 8):
                            emit_rot(k, wts[k])
            if DBG:
                nc.sync.dma_start(out=dbg_qt_d[:], in_=qt_sb[:])
                nc.sync.dma_start(out=dbg_v_d[:], in_=V_sb[:])
    nc.compile()
    return nc


def _host_prep(x, oft_r, codes, codebooks, scales, bias):
    """Shard + repack all inputs for the 8 cores."""
    xT = np.ascontiguousarray(
        np.asarray(x, dtype=np.float32).reshape(TOK, IN_F).astype(BF16).T
    )                                                           # [4096, 16384]
    codes2 = np.asarray(codes, dtype=np.int64)[:, :, 0]         # [4096, 512]
    cb = np.asarray(codebooks, dtype=np.float32)[0]             # [65536, 8]
    scales = np.asarray(scales, dtype=np.float32).reshape(OUT_F)
    bias = np.asarray(bias, dtype=np.float32).reshape(OUT_F)
    R = np.asarray(oft_r, dtype=np.float32)                     # [128, 32, 32]

    table = np.zeros((HALF_CB, ELEM), dtype=BF16)
    table[:, 0:GROUP] = cb[:HALF_CB].astype(BF16)
    table[:, GROUP:2 * GROUP] = cb[HALF_CB:].astype(BF16)

    rbd = np.zeros((N_IC, 128, 128), dtype=np.float32)
    Rb = R.reshape(N_IC, 4, 32, 32)
    for a in range(4):
        rbd[:, a * 32:(a + 1) * 32, a * 32:(a + 1) * 32] = Rb[:, a]
    identf = np.eye(128, dtype=np.float32)

    idx14 = (codes2 & 32767).astype(np.int16)
    mfull = (codes2 >> 15).astype(np.float32)

    s_g, gl_g, o_g = np.meshgrid(
        np.arange(4), np.arange(16), np.arange(128), indexing="ij"
    )
    c_gm = s_g * 16 + (o_g >> 5) * 4 + (gl_g & 3)
    p_gm = (gl_g >> 2) * 32 + (o_g & 31)
    n_g = (c_gm * 128 + p_gm).ravel()
    p_g = p_gm.ravel()
    c_g = c_gm.ravel()

    in_maps = []
    for r in range(N_CORES):
        idx_all = np.empty((N_CALLS, 128, NIDX // 16), dtype=np.int16)
        abt_all = np.empty((N_CALLS, 128, 64, 16), dtype=BF16)
        for k in range(N_CALLS):
            oc, j = k // 8, k % 8
            ic = 4 * j + s_g
            g = ic * 16 + gl_g
            o_glob = r * OUT_PC + oc * 128 + o_g
            vals = idx14[o_glob, g]
            stream = np.empty(NIDX, dtype=np.int16)
            stream[n_g] = vals.ravel()
            idx_all[k] = np.broadcast_to(
                stream.reshape(NIDX // 16, 16).T[None, :, :], (8, 16, NIDX // 16)
            ).reshape(128, NIDX // 16)
            sc = scales[o_glob]
            B = sc * mfull[o_glob, g]
            A = sc - B
            ab = np.empty((128, 64, 16), dtype=np.float32)
            ab[p_g, c_g, 0:8] = A.ravel()[:, None]
            ab[p_g, c_g, 8:16] = B.ravel()[:, None]
            abt_all[k] = ab.astype(BF16)
        bias_p = np.zeros((128, 4), dtype=np.float32)
        for oc in range(4):
            bias_p[:, oc] = bias[r * OUT_PC + oc * 128:r * OUT_PC + (oc + 1) * 128]
        in_maps.append(
            dict(
                xT=xT,
                table=table,
                idx=idx_all,
                abt=abt_all.reshape(N_CALLS, 128, 1024),
                rbd=rbd,
                identf=identf,
                bias_p=bias_p,
            )
        )
    return in_maps


def kernel(x, oft_r, codes, codebooks, scales, bias):
    global LAST_RESULT
    from concourse.bass_utils import run_bass_kernel_spmd

    if "nc" not in _BUILD_CACHE:
        _BUILD_CACHE["nc"] = _build_nc()
    nc = _BUILD_CACHE["nc"]

    in_maps = _host_prep(x, oft_r, codes, codebooks, scales, bias)
    trace = bool(int(os.environ.get("AQLM_TRACE", "0")))
    res = run_bass_kernel_spmd(nc, in_maps, core_ids=list(range(N_CORES)), trace=trace)
    LAST_RESULT = res

    out = np.empty((TOK, OUT_F), dtype=np.float32)
    for r in range(N_CORES):
        out[:, r * OUT_PC:(r + 1) * OUT_PC] = (
            res.results[r]["outT"].T.astype(np.float32)
        )
    return out.reshape(4, 4096, 4096).astype(np.asarray(x).dtype)


# revision 16
# speedup vs baseline: 1.1890x; 1.0259x over previous
"""AqlmOFTLinear distributed Trainium2 kernel (8 NeuronCores), v2.

Strategy (pure tensor-parallel over out_features, no collectives):
  - Each core owns 512 out-features: it dequants its own AQLM weight shard,
    rotates it (V = Q_blockdiag @ W^T) into SBUF, and computes
    out^T[o_shard, t] for ALL 16384 tokens.  Host concatenates shards.
  - AQLM dequant via gpsimd.dma_gather from a paired codebook table
    [32768, 128]bf16 (entry q = [cb[q] | cb[q+32768]]); int15 index.
    32 calls x 8192 idx, one SWDGE queue per call in oc-major bursts of
    4 -- the 4 queues generate descriptors in parallel (~64us/call,
    ~82us/burst), so V[oc] lands every ~165us and the whole gather
    stream finishes in ~660us.
  - The gather stream order is chosen so the (o, group) -> (i'', o)
    transpose needed for the rotation matmul is exactly a DVE 32x32
    stream-transpose (InstStreamTranspose) -- no PE transposes at all.
    Select+scale is one DVE multiply with host-built coefficients
    (lo: s*(1-m), hi: s*m) and the lo+hi sum is one DVE add.
  - OFT Cayley on-device: Q^T = (I+S^8)(I+S^4)(I+S^2)(I-S)^2 with the 128
    32x32 blocks packed 4-up into block-diagonal 128x128 f32 matmuls.
  - Main matmul in bf16, back-to-back 512-wide matmuls (measured ~260ns
    sustained, LDWEIGHTS hidden): PSUM holds 2 o-blocks x 1024 tokens;
    x is streamed from DRAM twice (once per o-block pair).  Bias fused
    into the PSUM->SBUF evacuation; bf16 output.
"""

import os
import sys

import numpy as np

sys.path.insert(0, "/opt/trn_rl_repo")

import ml_dtypes

BF16 = ml_dtypes.bfloat16

N_CORES = 8
IN_F = 4096
OUT_F = 4096
TOK = 16384
OUT_PC = OUT_F // N_CORES        # 512 out-features per core
GROUP = 8
N_IC = IN_F // 128               # 32 input-feature chunks
HALF_CB = 32768                  # paired table entries
ELEM = 128                       # bf16 elems per table entry (256B)
N_CALLS = 32                     # gather calls per core: (oc, j)
NIDX = 8192                      # indices per gather call (4 ic-chunks)
NQ = 4                           # SWDGE queues
TCH = 1024                       # tokens per main-GEMM chunk
N_CH = TOK // TCH                # 16 chunks per pass

_BUILD_CACHE = {}
LAST_RESULT = None


def _patched_dma_gather():
    """dma_gather with the elem_size %256 assert relaxed: the 256B constraint
    is xbar-transpose-only; natural-mode 32B elements work on HW (verified)
    and cut gather traffic 8x by skipping the table pad."""
    import inspect
    import re

    import concourse.bass as cb

    fsrc = inspect.getsource(type(cb.Bass().gpsimd).dma_gather)
    fsrc = fsrc.replace(
        "elem_size_bytes > 0 and elem_size_bytes % 256 == 0", "elem_size_bytes > 0"
    )
    fsrc = re.sub(r"^    def dma_gather", "def dma_gather", fsrc, flags=re.M)
    fsrc = re.sub(r"\n    ", "\n", fsrc)
    ns = dict(vars(cb))
    exec(compile(fsrc, "patched_dma_gather", "exec"), ns)
    return ns["dma_gather"]


def _build_nc():
    from concourse import bacc, mybir, tile

    dma_gather32 = _patched_dma_gather()

    f32 = mybir.dt.float32
    bf16 = mybir.dt.bfloat16
    i16 = mybir.dt.int16

    nc = bacc.Bacc(num_devices=N_CORES, num_swdge_queues=NQ)

    # ---- DRAM parameters ----
    xT_d = nc.declare_dram_parameter("xT", [IN_F, TOK], bf16, isOutput=False)
    table_d = nc.declare_dram_parameter("table", [HALF_CB, ELEM], bf16, isOutput=False)
    idx_d = nc.declare_dram_parameter("idx", [N_CALLS, 128, NIDX // 16], i16, isOutput=False)
    abt_d = nc.declare_dram_parameter("abt", [N_CALLS, 128, 1024], bf16, isOutput=False)
    rbd_d = nc.declare_dram_parameter("rbd", [N_IC, 128, 128], f32, isOutput=False)
    identf_d = nc.declare_dram_parameter("identf", [128, 128], f32, isOutput=False)
    bias_d = nc.declare_dram_parameter("bias_p", [128, 4], f32, isOutput=False)
    outT_d = nc.declare_dram_parameter("outT", [OUT_PC, TOK], bf16, isOutput=True)
    DBG = bool(int(os.environ.get("AQLM_DEBUG", "0")))
    if DBG:
        dbg_qt_d = nc.declare_dram_parameter("dbg_qt", [128, N_IC, 128], bf16, isOutput=True)
        dbg_v_d = nc.declare_dram_parameter("dbg_v", [128, N_IC, OUT_PC], bf16, isOutput=True)
        dbg_g_d = nc.declare_dram_parameter("dbg_g", [128, 64, 16], bf16, isOutput=True)
        dbg_gs_d = nc.declare_dram_parameter("dbg_gs", [128, 64, 16], bf16, isOutput=True)
        dbg_w4_d = nc.declare_dram_parameter("dbg_w4", [128, 512], bf16, isOutput=True)
        dbg_wt_d = nc.declare_dram_parameter("dbg_wt", [128, 512], bf16, isOutput=True)

    with tile.TileContext(nc) as tc:
        with (
            tc.tile_pool(name="const", bufs=1) as constp,
            tc.tile_pool(name="qt", bufs=1) as qtp,
            tc.tile_pool(name="vsb", bufs=1) as vp,
            tc.tile_pool(name="xh", bufs=5) as xhp,
            tc.tile_pool(name="cay", bufs=4) as cayp,
            tc.tile_pool(name="idx", bufs=12) as idxp,
            tc.tile_pool(name="abt", bufs=10) as abtp,
            tc.tile_pool(name="g", bufs=8) as gp,
            tc.tile_pool(name="gs", bufs=2) as gsp,
            tc.tile_pool(name="w4", bufs=2) as w4p,
            tc.tile_pool(name="wt", bufs=4) as wtp,
            tc.tile_pool(name="ob", bufs=4) as obp,
            tc.tile_pool(name="psm", bufs=3, space="PSUM") as psmp,
            tc.tile_pool(name="pss", bufs=2, space="PSUM") as pssp,
        ):
            # ---- constants (sync queue) ----
            identf = constp.tile([128, 128], f32)
            nc.sync.dma_start(out=identf[:], in_=identf_d[:])
            bias_sb = constp.tile([128, 4], f32)
            nc.sync.dma_start(out=bias_sb[:], in_=bias_d[:])
            ident4 = constp.tile([128, 4, 128], f32)
            for k in range(4):
                nc.vector.tensor_copy(ident4[:, k, :], identf[:])

            qt_sb = qtp.tile([128, N_IC, 128], bf16)   # Q^T block-diag chunks
            V_sb = vp.tile([128, N_IC, OUT_PC], bf16)  # rotated weight shard
            nidx_reg = nc.gpsimd.to_reg(NIDX)

            # ---- idx/abt prefetch (sync; bursts 4..7 woven into the main
            # loop) and gathers (gpsimd).  scalar queue = evacuations ONLY so
            # nothing gated ever blocks the PSUM drain.
            pf = {}
            g_tiles = {}

            def emit_prefetch(k):
                idx_sb = idxp.tile([128, NIDX // 16], i16, tag="idx")
                nc.sync.dma_start(out=idx_sb[:], in_=idx_d[k, :, :])
                abt_sb = abtp.tile([128, 64, 16], bf16, tag="abt")
                nc.sync.dma_start(
                    out=abt_sb[:],
                    in_=abt_d[k, :, :].rearrange("p (c e) -> p c e", e=16),
                )
                pf[k] = (idx_sb, abt_sb)

            def emit_gather(k):
                idx_sb, abt_sb = pf[k]
                G = gp.tile([128, 64, 16], bf16, tag="G")
                dma_gather32(
                    nc.gpsimd, G[:], table_d[:, 0:16], idx_sb[:],
                    num_idxs=NIDX, num_idxs_reg=nidx_reg,
                    elem_size=16, elem_step=ELEM,
                    single_packet=False, queue_num=k % NQ,
                )
                if DBG and k == 0:
                    nc.sync.dma_start(out=dbg_g_d[:], in_=G[:])
                g_tiles[k] = (G, abt_sb)

            for k in range(16):
                emit_prefetch(k)
            for k in range(16):
                emit_gather(k)

            # ================= Cayley (PE + DVE, head) =================
            for g in range(8):
                rbd_sb = cayp.tile([128, 4, 128], f32, tag="cay")
                nc.sync.dma_start(
                    out=rbd_sb[:],
                    in_=rbd_d[g * 4:(g + 1) * 4, :, :].rearrange("c p f -> p c f"),
                )
                psT = pssp.tile([128, 4, 128], f32, tag="sm")
                for k in range(4):
                    nc.tensor.transpose(psT[:, k, :], rbd_sb[:, k, :], identf[:])
                tmp = cayp.tile([128, 4, 128], f32, tag="cay")
                nc.vector.tensor_scalar_mul(tmp[:], rbd_sb[:], 0.5)
                S = cayp.tile([128, 4, 128], f32, tag="cay")
                nc.vector.scalar_tensor_tensor(
                    S[:], psT[:], -0.5, tmp[:],
                    mybir.AluOpType.mult, mybir.AluOpType.add,
                )
                negS = cayp.tile([128, 4, 128], f32, tag="cay")
                nc.vector.tensor_scalar_mul(negS[:], S[:], -1.0)
                P1T = cayp.tile([128, 4, 128], f32, tag="cay")  # I - S
                nc.vector.scalar_tensor_tensor(
                    P1T[:], S[:], -1.0, ident4[:],
                    mybir.AluOpType.mult, mybir.AluOpType.add,
                )
                P1 = cayp.tile([128, 4, 128], f32, tag="cay")  # I + S
                nc.vector.tensor_tensor(P1[:], S[:], ident4[:], mybir.AluOpType.add)
                ps2 = pssp.tile([128, 4, 128], f32, tag="sm")
                for k in range(4):
                    nc.tensor.matmul(ps2[:, k, :], negS[:, k, :], S[:, k, :])
                S2 = cayp.tile([128, 4, 128], f32, tag="cay")
                nc.vector.tensor_copy(S2[:], ps2[:])
                P2 = cayp.tile([128, 4, 128], f32, tag="cay")  # I + S^2
                nc.vector.tensor_tensor(P2[:], S2[:], ident4[:], mybir.AluOpType.add)
                ps4 = pssp.tile([128, 4, 128], f32, tag="sm")
                for k in range(4):
                    nc.tensor.matmul(ps4[:, k, :], S2[:, k, :], S2[:, k, :])
                S4 = cayp.tile([128, 4, 128], f32, tag="cay")
                nc.vector.tensor_copy(S4[:], ps4[:])
                P3 = cayp.tile([128, 4, 128], f32, tag="cay")  # I + S^4
                nc.vector.tensor_tensor(P3[:], S4[:], ident4[:], mybir.AluOpType.add)
                ps8 = pssp.tile([128, 4, 128], f32, tag="sm")
                for k in range(4):
                    nc.tensor.matmul(ps8[:, k, :], S4[:, k, :], S4[:, k, :])
                P4 = cayp.tile([128, 4, 128], f32, tag="cay")  # I + S^8
                nc.vector.scalar_tensor_tensor(
                    P4[:], ps8[:], 1.0, ident4[:],
                    mybir.AluOpType.mult, mybir.AluOpType.add,
                )
                psT1 = pssp.tile([128, 4, 128], f32, tag="sm")
                for k in range(4):
                    nc.tensor.matmul(psT1[:, k, :], P1[:, k, :], P1T[:, k, :])
                T1 = cayp.tile([128, 4, 128], f32, tag="cay")  # (I-S)^2
                nc.vector.tensor_copy(T1[:], psT1[:])
                psb1 = pssp.tile([128, 4, 128], f32, tag="sm")
                for k in range(4):
                    nc.tensor.matmul(psb1[:, k, :], P2[:, k, :], T1[:, k, :])
                B1 = cayp.tile([128, 4, 128], f32, tag="cay")
                nc.vector.tensor_copy(B1[:], psb1[:])
                psb2 = pssp.tile([128, 4, 128], f32, tag="sm")
                for k in range(4):
                    nc.tensor.matmul(psb2[:, k, :], P3[:, k, :], B1[:, k, :])
                B2 = cayp.tile([128, 4, 128], f32, tag="cay")
                nc.vector.tensor_copy(B2[:], psb2[:])
                psb3 = pssp.tile([128, 4, 128], f32, tag="sm")
                for k in range(4):
                    nc.tensor.matmul(psb3[:, k, :], P4[:, k, :], B2[:, k, :])
                nc.vector.tensor_copy(qt_sb[:, g * 4:(g + 1) * 4, :], psb3[:])

            # ---- dequant per call: DVE mul/add/stream-transpose, PE rot ----
            def emit_dequant(k):
                G, abt_sb = g_tiles[k]
                Gs = gsp.tile([128, 64, 16], bf16, tag="gs")
                nc.vector.tensor_tensor(Gs[:], G[:], abt_sb[:], mybir.AluOpType.mult)
                # c-slot mapping (s, oh, gm) makes the lo+hi add output plain
                # contiguous; the stream-transpose then sees [p, 16 tiles, 32]
                # (3-dim AP limit) and lands W^T tiles [i''-local, o-local].
                W4 = w4p.tile([128, 512], bf16, tag="w4")
                nc.vector.tensor_tensor(
                    W4[:].rearrange("p (c j) -> p c j", j=8),
                    Gs[:, :, 0:8], Gs[:, :, 8:16], mybir.AluOpType.add,
                )
                wt = wtp.tile([128, 512], bf16, tag="wt")
                nc.vector.transpose(
                    wt[:].rearrange("p (t f) -> p t f", f=32),
                    W4[:].rearrange("p (t f) -> p t f", f=32),
                )
                if DBG and k == 0:
                    nc.sync.dma_start(out=dbg_gs_d[:], in_=Gs[:])
                    nc.sync.dma_start(out=dbg_w4_d[:], in_=W4[:])
                    nc.sync.dma_start(out=dbg_wt_d[:], in_=wt[:])
                return wt

            def emit_rot(k, wt):
                oc, j = k // 8, k % 8
                psv = pssp.tile([128, 4, 128], f32, tag="sm")
                for s in range(4):
                    ic = 4 * j + s
                    nc.tensor.matmul(
                        psv[:, s, :], qt_sb[:, ic, :], wt[:, s * 128:(s + 1) * 128],
                        start=True, stop=True,
                    )
                nc.vector.tensor_copy(
                    V_sb[:, 4 * j:4 * j + 4, oc * 128:(oc + 1) * 128], psv[:]
                )

            # DVE dequant for calls 0..15 up-front (gathers pace them);
            # calls 16..31 are prefetched+gathered+dequanted at weave points.
            wts = {}
            for k in range(16):
                wts[k] = emit_dequant(k)
            for k in range(8):
                emit_rot(k, wts[k])

            def emit_chunk(ocA, ocB, c, xh_quarters, only=None):
                obs = ((0, ocA), (1, ocB))
                if only is not None:
                    obs = (obs[only],)
                for obi, oc in obs:
                    psm = psmp.tile([128, TCH], f32, tag="m")
                    for ic in range(N_IC):
                        xh = xh_quarters[ic // 8]
                        for h2 in range(2):
                            nc.tensor.matmul(
                                psm[:, h2 * 512:(h2 + 1) * 512],
                                V_sb[:, ic, oc * 128:(oc + 1) * 128],
                                xh[:, ic % 8, h2 * 512:(h2 + 1) * 512],
                                start=(ic == 0), stop=(ic == N_IC - 1),
                            )
                    ob = obp.tile([128, TCH], bf16, tag="ob")
                    nc.scalar.activation(
                        ob[:], psm[:], mybir.ActivationFunctionType.Identity,
                        bias=bias_sb[:, oc:oc + 1],
                    )
                    nc.sync.dma_start(
                        out=outT_d[oc * 128:(oc + 1) * 128, c * TCH:(c + 1) * TCH],
                        in_=ob[:],
                    )

            for p, (ocA, ocB) in enumerate(((0, 1), (2, 3))):
                for c in range(N_CH):
                    quarters = []
                    for ih in range(4):
                        xh = xhp.tile([128, 8, TCH], bf16, tag="xh")
                        nc.sync.dma_start(
                            out=xh[:],
                            in_=xT_d[
                                ih * 1024:(ih + 1) * 1024, c * TCH:(c + 1) * TCH
                            ].rearrange("(ic q) t -> q ic t", q=128),
                        )
                        quarters.append(xh)
                    if p == 0 and c == 0:
                        # obA first so it is not blocked behind oc1 rotations
                        emit_chunk(ocA, ocB, c, quarters, only=0)
                        for k in range(8, 16):
                            emit_rot(k, wts[k])
                        emit_chunk(ocA, ocB, c, quarters, only=1)
                    else:
                        emit_chunk(ocA, ocB, c, quarters)
                    if p == 0 and c in (4, 7, 9, 12):
                        # PE rotations for oc2/oc3, paced to gather bursts
                        base = {4: 16, 7: 20, 9: 24, 12: 28}[c]
                        for k in range(base, base + 4):
                            emit_rot(k, wts[k])
                    if p == 0 and c in (1, 3, 5, 7):
                        # prefetch + gather + DVE dequant for bursts 4..7
                        base = 16 + 4 * ((c - 1) // 2)
                        for k in range(base, base + 4):
                            emit_prefetch(k)
                        for k in range(base, base + 4):
                            emit_gather(k)
                        for k in range(base, base + 4):
                            wts[k] = emit_dequant(k)
            if DBG:
                nc.sync.dma_start(out=dbg_qt_d[:], in_=qt_sb[:])
                nc.sync.dma_start(out=dbg_v_d[:], in_=V_sb[:])
    nc.compile()
    return nc


def _host_prep(x, oft_r, codes, codebooks, scales, bias):
    """Shard + repack all inputs for the 8 cores."""
    xT = np.ascontiguousarray(
        np.asarray(x, dtype=np.float32).reshape(TOK, IN_F).astype(BF16).T
    )                                                           # [4096, 16384]
    codes2 = np.asarray(codes, dtype=np.int64)[:, :, 0]         # [4096, 512]
    cb = np.asarray(codebooks, dtype=np.float32)[0]             # [65536, 8]
    scales = np.asarray(scales, dtype=np.float32).reshape(OUT_F)
    bias = np.asarray(bias, dtype=np.float32).reshape(OUT_F)
    R = np.asarray(oft_r, dtype=np.float32)                     # [128, 32, 32]

    table = np.zeros((HALF_CB, ELEM), dtype=BF16)
    table[:, 0:GROUP] = cb[:HALF_CB].astype(BF16)
    table[:, GROUP:2 * GROUP] = cb[HALF_CB:].astype(BF16)

    rbd = np.zeros((N_IC, 128, 128), dtype=np.float32)
    Rb = R.reshape(N_IC, 4, 32, 32)
    for a in range(4):
        rbd[:, a * 32:(a + 1) * 32, a * 32:(a + 1) * 32] = Rb[:, a]
    identf = np.eye(128, dtype=np.float32)

    idx14 = (codes2 & 32767).astype(np.int16)
    mfull = (codes2 >> 15).astype(np.float32)

    s_g, gl_g, o_g = np.meshgrid(
        np.arange(4), np.arange(16), np.arange(128), indexing="ij"
    )
    c_gm = s_g * 16 + (o_g >> 5) * 4 + (gl_g & 3)
    p_gm = (gl_g >> 2) * 32 + (o_g & 31)
    n_g = (c_gm * 128 + p_gm).ravel()
    p_g = p_gm.ravel()
    c_g = c_gm.ravel()

    in_maps = []
    for r in range(N_CORES):
        idx_all = np.empty((N_CALLS, 128, NIDX // 16), dtype=np.int16)
        abt_all = np.empty((N_CALLS, 128, 64, 16), dtype=BF16)
        for k in range(N_CALLS):
            oc, j = k // 8, k % 8
            ic = 4 * j + s_g
            g = ic * 16 + gl_g
            o_glob = r * OUT_PC + oc * 128 + o_g
            vals = idx14[o_glob, g]
            stream = np.empty(NIDX, dtype=np.int16)
            stream[n_g] = vals.ravel()
            idx_all[k] = np.broadcast_to(
                stream.reshape(NIDX // 16, 16).T[None, :, :], (8, 16, NIDX // 16)
            ).reshape(128, NIDX // 16)
            sc = scales[o_glob]
            B = sc * mfull[o_glob, g]
            A = sc - B
            ab = np.empty((128, 64, 16), dtype=np.float32)
            ab[p_g, c_g, 0:8] = A.ravel()[:, None]
            ab[p_g, c_g, 8:16] = B.ravel()[:, None]
            abt_all[k] = ab.astype(BF16)
        bias_p = np.zeros((128, 4), dtype=np.float32)
        for oc in range(4):
            bias_p[:, oc] = bias[r * OUT_PC + oc * 128:r * OUT_PC + (oc + 1) * 128]
        in_maps.append(
            dict(
                xT=xT,
                table=table,
                idx=idx_all,
                abt=abt_all.reshape(N_CALLS, 128, 1024),
                rbd=rbd,
                identf=identf,
                bias_p=bias_p,
            )
        )
    return in_maps


def kernel(x, oft_r, codes, codebooks, scales, bias):
    global LAST_RESULT
    from concourse.bass_utils import run_bass_kernel_spmd

    if "nc" not in _BUILD_CACHE:
        _BUILD_CACHE["nc"] = _build_nc()
    nc = _BUILD_CACHE["nc"]

    in_maps = _host_prep(x, oft_r, codes, codebooks, scales, bias)
    trace = bool(int(os.environ.get("AQLM_TRACE", "0")))
    res = run_bass_kernel_spmd(nc, in_maps, core_ids=list(range(N_CORES)), trace=trace)
    LAST_RESULT = res

    out = np.empty((TOK, OUT_F), dtype=np.float32)
    for r in range(N_CORES):
        out[:, r * OUT_PC:(r + 1) * OUT_PC] = (
            res.results[r]["outT"].T.astype(np.float32)
        )
    return out.reshape(4, 4096, 4096).astype(np.asarray(x).dtype)
